# revision 58
# baseline (speedup 1.0000x reference)
"""Multi-head attention forward on 8 Trainium2 NeuronCores (Bass/Tile).

Problem: nn_MultiHeadAttention — B=8, T=1024, C=768, H=12, D=64, fp32.

Sharding: data-parallel over batch — B=8 -> one batch element per core; weights
broadcast to all cores, no collectives. The host pre-transposes x[b] to x^T
[C, T], converts x/W to bf16, and pre-arranges biases; the full output is
gathered by stacking per-core results.

All matmul operands are bf16: same PE rate as float32r (matmul cost is output
free-size columns/cycle, independent of K and partition count) but half the
DMA bytes and SBUF footprint. DMA is what gates the startup (serial at
~352GB/s with ~625ns HWDGE issue per copy), and the SBUF slack lets Wp load
up front into its own buffer. End-to-end rel err 6.4e-3 vs the f32 reference
(tolerance 2e-2): psum accumulation, bias adds, and softmax normalization all
stay f32.

Structure per core (PE floor ~143us, total ~162us sim):
  1. V = x @ Wv into V_aug [128, T/128, H, 65] with a ones column per head
     (the ones row of P@V_aug yields the softmax denominator for free).
     Emitted k-OUTER over 4-ts blocks (psum: 2 ts in psQ + 2 ts in psS) so PE
     consumption (~1.2us/k) matches the per-k (Wv, xT) DMA arrival (~1.3us) —
     a ts-outer loop would stall the in-order PE waiting for the k=5 chunk.
  2. Per pair p: Q^T [128, T] (head-major transposed — exactly what the S^T
     matmul needs) and K^T as two zero-padded tiles KTz[hh] [128, T]; S^T
     contracts the full K=128 partitions (a K=64 matmul runs at half PE rate
     on hw; zero-padding restores full rate at identical results). The zero
     halves are invariant across pairs and memset ONCE.
  3. Per head, per key-chunk j: S^T[j] psum [128,1024]; P = exp(S^T/8) on
     ScalarE (f32 psum -> bf16 sbuf; no max subtraction needed: logits are
     ~N(0,1) so exp is safely in range); Ytil[65, i*512] accumulates
     matmul(V_aug[:, j, h, :], P chunk) over j. The chunk loop is
     software-pipelined depth 2 — S^T(j+1) is emitted between S^T(j) and
     P@V(j) — so the ~1us exp latency never stalls the in-order PE, and the
     NEXT pair's projection k-steps are emitted as fillers inside the loop
     (Q^T steps fill head 2p, K^T steps fill head 2p+1; the bias-add
     finalizers ride the 7th filler slot so they are on the DVE queue before
     the next head's normalize). The last pair, with no projections left,
     pre-accumulates the output projection's ts0 k<=4 steps instead (YT's
     k-row holds pair k's heads, so those rows are final by then).
  4. Normalize: one wide [65,512] psum->sbuf copy releases the psum bank
     early (DVE cost is free-size only, so it costs the same as a [1,512]
     copy), then a single-input sbuf copy moves the denominator row to
     partition 0, reciprocal_approx_fast + GpSimd partition_broadcast +
     multiply into Y^T [C, T] bf16 — all off the PE critical path.
     Hardware AP rules found the hard way: reciprocal_approx_fast needs a
     base-partition-0 input; two-input SBUF ops need equal base partitions;
     partition ranges must not cross the 64 boundary (so offsets 0/64 only).
  5. out = Y^T.T @ Wp + bp -> f32, per 128-row tile: all six 512-wide
     k-matmuls first, then the 256-wide ones, so the big bias-add overlaps
     the tail matmuls; adds+DMA split per 512/256-col half so the final
     transfer is small.

DMA order: xT0 first half + Wv0 first (the only data the first V matmuls
need; HWDGE issues serially at ~625ns/copy so every early DMA delays all
later ones), remaining (xT[k], Wv[k]) pairs, xT0 second half, bvB, tiny
biases, pair-0 Wq/Wk column slices, remaining per-pair Wq/Wk slices
([C,128] strided loads, 512B lines), Wp, bpB. Per-pair W slices mean pair-0
attention is not gated on the full 4.5MB of Wq/Wk.

A matmul output may not span a psum bank (max 512 f32 columns per matmul).
"""
import numpy as np

B, T, C = 8, 1024, 768
H, D = 12, 64
P = 128
KS = C // P          # 6 contraction subtiles
TS = T // P          # 8 t subtiles
NI = T // 512        # 2 i-chunks of 512
N_CORES = 8

_RUNNER_CACHE = {}


def build_nc(reps: int = 1, phases: int = 4, variant: str = "full"):
    import concourse.bacc as bacc
    import concourse.mybir as mybir
    import concourse.tile as tile
    from contextlib import ExitStack

    f32 = mybir.dt.float32
    bf16 = mybir.dt.bfloat16
    if variant.startswith("f32r"):
        bf16 = mybir.dt.float32r
    AF = mybir.ActivationFunctionType
    ALU = mybir.AluOpType

    nc = bacc.Bacc(num_devices=N_CORES)

    xT_d = nc.dram_tensor("xT", [C, T], bf16, kind="ExternalInput")
    W_d = {w: nc.dram_tensor(f"W{w}", [C, C], bf16, kind="ExternalInput")
           for w in ("q", "k", "v", "p")}
    bqT_d = nc.dram_tensor("bqT", [P, KS], f32, kind="ExternalInput")
    bkT_d = nc.dram_tensor("bkT", [P, KS], f32, kind="ExternalInput")
    bvB_d = nc.dram_tensor("bvB", [1, C], f32, kind="ExternalInput")
    bpB_d = nc.dram_tensor("bpB", [P, C], f32, kind="ExternalInput")
    y_d = nc.dram_tensor("y", [T, C], f32, kind="ExternalOutput")
    dbg = {}
    if variant == "debug":
        dbg["Vdbg"] = nc.dram_tensor("Vdbg", [P, TS * H * (D + 1)], bf16,
                                     kind="ExternalOutput")
        dbg["QTdbg"] = nc.dram_tensor("QTdbg", [P, T], bf16,
                                      kind="ExternalOutput")
        dbg["KTdbg"] = nc.dram_tensor("KTdbg", [P, 2 * T], bf16,
                                      kind="ExternalOutput")
        dbg["PTdbg"] = nc.dram_tensor("PTdbg", [P, T], bf16,
                                      kind="ExternalOutput")
        dbg["YTdbg"] = nc.dram_tensor("YTdbg", [P, KS * T], bf16,
                                      kind="ExternalOutput")

    with tile.TileContext(nc) as tc, ExitStack() as ctx:
        const = ctx.enter_context(tc.tile_pool(name="const", bufs=1))
        ppool = ctx.enter_context(tc.tile_pool(name="pt", bufs=6))
        npool = ctx.enter_context(tc.tile_pool(name="norm", bufs=8))
        opool = ctx.enter_context(tc.tile_pool(name="out", bufs=3))
        psQ = ctx.enter_context(tc.tile_pool(name="psQ", bufs=4, space="PSUM"))
        psS = ctx.enter_context(tc.tile_pool(name="psS", bufs=2, space="PSUM"))

        def body(_iv=None):
            # ---- loads ----
            xTr = const.tile([P, KS, T], bf16, tag="xT", name="xTr")
            Wr = {}
            for w in ("q", "k", "v", "p"):
                Wr[w] = const.tile([P, KS, C], bf16, tag=f"W{w}", name=f"W{w}r")
            xT_r = xT_d.rearrange("(ks p) t -> p ks t", p=P)
            W_r = {w: W_d[w].rearrange("(ks p) c -> p ks c", p=P)
                   for w in ("q", "k", "v", "p")}
            # HWDGE issues serially at ~625ns/copy, so the V-phase stream
            # (xT[k], Wv[k] per k, x first) gets the front slots exclusively;
            # every extra early DMA delays ALL later issues by 625ns. The
            # tiny bias loads ride after (first consumer is ~18us in).
            bvB = const.tile([P, C], f32, tag="bvB", name="bvB")
            bpB = const.tile([P, C], f32, tag="bpB", name="bpB")
            nc.sync.dma_start(xTr[:, 0, 0:512], xT_r[:, 0, 0:512])
            nc.sync.dma_start(Wr["v"][:, 0, :], W_r["v"][:, 0, :])
            for k in range(1, KS):
                nc.sync.dma_start(xTr[:, k, :], xT_r[:, k, :])
                nc.sync.dma_start(Wr["v"][:, k, :], W_r["v"][:, k, :])
            # bv is 3KB of real data: load one partition row and replicate
            # on the (idle) Pool engine — the full [128,C] broadcast load was
            # a 1.1us transfer gating V block A's bias adds (whose psum the
            # B block WARs on)
            bv1 = const.tile([1, C], f32, tag="bv1", name="bv1")
            nc.sync.dma_start(bv1[:], bvB_d[:, :])
            nc.sync.dma_start(xTr[:, 0, 512:1024], xT_r[:, 0, 512:1024])
            nc.gpsimd.partition_broadcast(bvB[:], bv1[0:1, :])
            bqT = const.tile([P, KS], f32, tag="bqT", name="bqT")
            nc.sync.dma_start(bqT[:], bqT_d[:, :])
            bkT = const.tile([P, KS], f32, tag="bkT", name="bkT")
            nc.sync.dma_start(bkT[:], bkT_d[:, :])
            if variant == "fullkW":
                for k in range(KS):
                    nc.sync.dma_start(Wr["q"][:, k, :], W_r["q"][:, k, :])
                for k in range(KS):
                    nc.sync.dma_start(Wr["k"][:, k, :], W_r["k"][:, k, :])
            else:
                nc.sync.dma_start(Wr["q"][:, :, 0:P], W_r["q"][:, :, 0:P])
                nc.sync.dma_start(Wr["k"][:, :, 0:P], W_r["k"][:, :, 0:P])
                for pp in range(1, KS):
                    sl = slice(pp * P, (pp + 1) * P)
                    nc.sync.dma_start(Wr["q"][:, :, sl], W_r["q"][:, :, sl])
                    nc.sync.dma_start(Wr["k"][:, :, sl], W_r["k"][:, :, sl])
            nc.sync.dma_start(Wr["p"][:], W_r["p"][:])
            nc.sync.dma_start(bpB[:], bpB_d[:, :])
            ones1 = const.tile([P, 1], f32, tag="ones", name="ones1")
            nc.vector.memset(ones1[:], 1.0)
            if phases < 4:
                YTdummy = opool.tile([P, C], f32, tag="ot", name="ytd")
                nc.vector.memset(YTdummy[:], 0.0)
                nc.sync.dma_start(y_d[0:P, :], YTdummy[:])

            # ---- V (k-outer, 4-ts blocks) into V_aug with ones column ----
            # V_aug column layout [ones | 31 zeros | V(64)]: the P@V ones-row
            # (softmax denominator) lands at psy partition 0 where the hw
            # reciprocal needs it (its input AP must have no partition
            # offset), and V rows land at partition 32 (offsets must be
            # multiples of 32). The zero pad costs nothing: matmul time is
            # rhs-free-size only.
            VW = D + 1
            V_aug = const.tile([P, TS, H, VW], bf16, tag="Vaug", name="Vaug")
            nc.vector.tensor_copy(V_aug[:, :, :, D:D + 1],
                                  ones1[:].to_broadcast([P, TS, H, 1]))
            for blk in range(2):
                t0 = blk * 4
                psq2 = [[psQ.tile([P, 512], f32, tag="ps512", name="psq")
                         for _ in range(2)] for _ in range(2)]
                pss2 = [psS.tile([P, 1024], f32, tag="psS", name="pssv")
                        for _ in range(2)]
                # block B consumes k=0 LAST: its xT0 half is deliberately
                # loaded after the main stream (accumulation order is free)
                korder = list(range(KS)) if blk == 0 else list(range(1, KS)) + [0]
                for ki, k in enumerate(korder):
                    st, sp = (ki == 0), (ki == KS - 1)
                    for tt in range(2):
                        lhsT = xTr[:, k, (t0 + tt) * P:(t0 + tt + 1) * P]
                        nc.tensor.matmul(psq2[tt][0][:], lhsT,
                                         Wr["v"][:, k, 0:512], start=st, stop=sp)
                        nc.tensor.matmul(psq2[tt][1][:, 0:256], lhsT,
                                         Wr["v"][:, k, 512:768], start=st, stop=sp)
                    for tt in range(2):
                        lhsT = xTr[:, k, (t0 + 2 + tt) * P:(t0 + 3 + tt) * P]
                        # a matmul output may not span a psum bank: split 512+256
                        nc.tensor.matmul(pss2[tt][:, 0:512], lhsT,
                                         Wr["v"][:, k, 0:512], start=st, stop=sp)
                        nc.tensor.matmul(pss2[tt][:, 512:768], lhsT,
                                         Wr["v"][:, k, 512:768], start=st, stop=sp)
                for tt in range(2):
                    nc.vector.tensor_tensor(
                        V_aug[:, t0 + tt, 0:8, 0:D],
                        psq2[tt][0][:].rearrange("p (h d) -> p h d", h=8),
                        bvB[:, 0:512].rearrange("p (h d) -> p h d", h=8),
                        op=ALU.add)
                    nc.vector.tensor_tensor(
                        V_aug[:, t0 + tt, 8:12, 0:D],
                        psq2[tt][1][:, 0:256].rearrange("p (h d) -> p h d", h=4),
                        bvB[:, 512:768].rearrange("p (h d) -> p h d", h=4),
                        op=ALU.add)
                for tt in range(2):
                    nc.vector.tensor_tensor(
                        V_aug[:, t0 + 2 + tt, :, 0:D],
                        pss2[tt][:, 0:768].rearrange("p (h d) -> p h d", h=12),
                        bvB[:, 0:768].rearrange("p (h d) -> p h d", h=12),
                        op=ALU.add)

            if phases < 3:
                return
            if variant == "debug":
                nc.sync.dma_start(
                    dbg["Vdbg"][:, :],
                    V_aug[:].rearrange("p ts h d -> p (ts h d)"))

            # ---- attention with interleaved next-pair projections ----
            YT = const.tile([P, KS, T], bf16, tag="YTs", name="YT")
            # KTz zero halves are invariant across pairs — memset once
            KTzb = {(hh, par): const.tile([P, T], bf16, tag=f"KTz{hh}_{par}",
                                          name="KTz")
                    for hh in range(2) for par in range(2)}
            if variant == "safezero":
                z64 = const.tile([64, T], f32, tag="z64", name="z64")
                nc.vector.memset(z64[:], 0.0)
                for par in range(2):
                    nc.scalar.copy(KTzb[(0, par)][64:128, :], z64[:])
                    nc.scalar.copy(KTzb[(1, par)][0:64, :], z64[:])
            else:
                for par in range(2):
                    nc.vector.memset(KTzb[(0, par)][64:128, :], 0.0)
                    nc.vector.memset(KTzb[(1, par)][0:64, :], 0.0)

            def make_proj(w, p):
                """Q^T/K^T projection for pair p: 6 k-step closures (2 matmuls
                each) plus a finalize closure (bias add)."""
                state = {}

                def step(k):
                    def run():
                        if k == 0:
                            state["ps"] = [psQ.tile([P, 512], f32, tag="ps512",
                                                    name="psq")
                                           for _ in range(NI)]
                        lhsT = Wr[w][:, k, p * P:(p + 1) * P]
                        for ti in range(NI):
                            nc.tensor.matmul(
                                state["ps"][ti][:], lhsT,
                                xTr[:, k, ti * 512:(ti + 1) * 512],
                                start=(k == 0), stop=(k == KS - 1))
                    return run

                steps = [step(k) for k in range(KS)]

                def finalize_q():
                    QTp = const.tile([P, T], bf16, tag=f"QT{p % 2}", name="QTp")
                    for ti in range(NI):
                        nc.vector.tensor_tensor(
                            QTp[:, ti * 512:(ti + 1) * 512], state["ps"][ti][:],
                            bqT[:, p:p + 1].to_broadcast([P, 512]), op=ALU.add)
                    return QTp

                def finalize_k():
                    KTz = {hh: KTzb[(hh, p % 2)] for hh in range(2)}
                    for ti in range(NI):
                        sl = slice(ti * 512, (ti + 1) * 512)
                        nc.vector.tensor_tensor(
                            KTz[0][0:64, sl], state["ps"][ti][0:64, :],
                            bkT[0:64, p:p + 1].to_broadcast([64, 512]), op=ALU.add)
                        nc.vector.tensor_tensor(
                            KTz[1][64:128, sl], state["ps"][ti][64:128, :],
                            bkT[64:128, p:p + 1].to_broadcast([64, 512]), op=ALU.add)
                    return KTz

                return steps, (finalize_q if w == "q" else finalize_k)

            def emit_head(p, hh, QTp, KTz, filler, fstart=0, direct=False):
                """Chunk loop software-pipelined depth 2: S(j+1) is emitted
                between S(j)'s exp and P@V(j) so exp latency is hidden from
                the in-order PE; one projection k-step filler per chunk."""
                h = 2 * p + hh
                b0 = 64 * hh
                psy = [psQ.tile([P, 512], f32, tag="ps512", name="psy")
                       for _ in range(NI)]
                pss_t = [None] * TS
                pt_t = [None] * TS

                def emit_S(j):
                    pss_t[j] = psS.tile([P, 1024], f32, tag="psS", name="pss")
                    for i in range(NI):
                        nc.tensor.matmul(
                            pss_t[j][:, i * 512:(i + 1) * 512],
                            KTz[hh][:, j * P:(j + 1) * P],
                            QTp[:, i * 512:(i + 1) * 512],
                            start=True, stop=True)
                    pt_t[j] = ppool.tile([P, 1024], bf16, tag="pt", name="pt")
                    nc.scalar.activation(pt_t[j][:], pss_t[j][:], AF.Exp,
                                         scale=0.125)

                emit_S(0)
                if variant == "debug" and h == 0:
                    nc.sync.dma_start(dbg["PTdbg"][:, :], pt_t[0][:])
                for j in range(TS):
                    if j + 1 < TS:
                        emit_S(j + 1)
                        if filler and j >= fstart:
                            filler.pop(0)()
                    for i in range(NI):
                        nc.tensor.matmul(
                            psy[i][0:VW, :], V_aug[:, j, h, :],
                            pt_t[j][:, i * 512:(i + 1) * 512],
                            start=(j == 0), stop=(j == TS - 1))
                while filler:
                    filler.pop(0)()
                # normalize: y^T = Ytil[0:64] * recip(Ytil[64]). Normally a
                # psum->sbuf copy goes first (frees the bank for the next
                # head's projections at the same DVE cost — priced by free
                # size); the last pair reads psum directly instead, which
                # shortens the chain the output projection waits on.
                # (reciprocal_approx_fast misbehaves on hw when its input AP
                # has a partition offset — keep dd at partition 0)
                # hw AP rules: reciprocal_approx_fast needs a base-0 SBUF
                # input; two-input SBUF ops need equal base partitions;
                # partition ranges can't cross the 64 boundary. Normally one
                # wide psum->sbuf copy releases the bank early and a
                # single-input sbuf copy (on the idle Pool engine) moves the
                # denominator row to partition 0. The LAST head skips the
                # wide copy and reads psum directly (psum operands are exempt
                # from the equal-base rule) — nothing reuses its bank, and it
                # shortens the chain the output projection waits on.
                for i in range(NI):
                    dd = npool.tile([1, 512], f32, tag="dd", name="dd")
                    yt = npool.tile([VW, 512], f32, tag=f"yt{i}", name="yt")
                    if direct and i == 0:
                        # last head, first half: nothing downstream needs its
                        # psum bank soon, so start the reciprocal chain
                        # straight from psum and let the release-copy overlap
                        # the Pool broadcast — shortens the YT chain the
                        # output projection's first tile waits on. The second
                        # half keeps copy-first: its psum buffer is what the
                        # out-projection's ts1 tile reuses.
                        nc.vector.tensor_copy(dd[0:1, :], psy[i][D:D + 1, :])
                    else:
                        nc.vector.tensor_copy(yt[:], psy[i][0:VW, :])
                        nc.vector.tensor_copy(dd[0:1, :], yt[D:D + 1, :])
                    rr = npool.tile([1, 512], f32, tag="rr", name="rr")
                    nc.vector.reciprocal_approx_fast(rr[0:1, :], dd[0:1, :])
                    rb = npool.tile([D, 512], f32, tag="rb", name="rb")
                    if direct and i == 0:
                        nc.vector.tensor_copy(yt[:], psy[i][0:VW, :])
                    nc.gpsimd.partition_broadcast(rb[:], rr[0:1, :])
                    nc.vector.tensor_tensor(
                        YT[b0:b0 + 64, p, i * 512:(i + 1) * 512],
                        yt[0:D, :], rb[:], op=ALU.mult)

            qsteps, qfin = make_proj("q", 0)
            for s in qsteps:
                s()
            QTp = qfin()
            ksteps, kfin = make_proj("k", 0)
            for s in ksteps:
                s()
            KTz = kfin()
            if variant == "debug":
                nc.sync.dma_start(dbg["QTdbg"][:, :], QTp[:])
                nc.sync.dma_start(dbg["KTdbg"][:, 0:T], KTz[0][:])
                nc.sync.dma_start(dbg["KTdbg"][:, T:2 * T], KTz[1][:])
            for p in range(KS):
                nxt = {}
                if p + 1 < KS:
                    nq, nqf = make_proj("q", p + 1)
                    nk, nkf = make_proj("k", p + 1)
                    # finalizers ride the 7th filler slot so the bias adds
                    # are on the DVE queue BEFORE the next head's normalize
                    # (the next pair's first S^T depends on them)
                    nq = nq + [lambda: nxt.__setitem__("QT", nqf())]
                    nk = nk + [lambda: nxt.__setitem__("KTz", nkf())]
                else:
                    nq, nk = [], []
                last = p == KS - 1
                if last:
                    # fill pair-5's chunks with output-projection k-steps for
                    # ts0: YT's k-row holds pair k's heads, so k<=4 rows are
                    # final before pair 5 runs (only k=5 must wait)
                    po0 = {}

                    def postep(k):
                        def run():
                            if k == 0:
                                po0["ps"] = [
                                    psQ.tile([P, 512], f32, tag="ps512",
                                             name="psq"),
                                    psQ.tile([P, 512], f32, tag="ps512",
                                             name="psq")]
                            lhsT = YT[:, k, 0:P]
                            nc.tensor.matmul(po0["ps"][0][:], lhsT,
                                             Wr["p"][:, k, 0:512],
                                             start=(k == 0), stop=False)
                            nc.tensor.matmul(po0["ps"][1][:, 0:256], lhsT,
                                             Wr["p"][:, k, 512:768],
                                             start=(k == 0), stop=False)
                        return run

                    nq = [postep(k) for k in range(KS - 1)]
                emit_head(p, 0, QTp, KTz, nq, fstart=0, direct=False)
                emit_head(p, 1, QTp, KTz, nk, fstart=0, direct=last)
                if not last:
                    QTp, KTz = nxt["QT"], nxt["KTz"]

            if variant == "debug":
                nc.sync.dma_start(
                    dbg["YTdbg"][:, :],
                    YT[:].rearrange("p ks t -> p (ks t)"))
            if phases < 4:
                return

            # ---- output projection (ts0's k<=4 pre-accumulated above) ----
            for ts_ in range(TS):
                if ts_ == 0:
                    po_ = po0["ps"]
                    krange = range(KS - 1, KS)
                else:
                    po_ = [psQ.tile([P, 512], f32, tag="ps512", name="psq")
                           for _ in range(2)]
                    krange = range(KS)
                for k in krange:
                    lhsT = YT[:, k, ts_ * P:(ts_ + 1) * P]
                    nc.tensor.matmul(po_[0][:], lhsT, Wr["p"][:, k, 0:512],
                                     start=(k == 0), stop=(k == KS - 1))
                for k in krange:
                    lhsT = YT[:, k, ts_ * P:(ts_ + 1) * P]
                    nc.tensor.matmul(po_[1][:, 0:256], lhsT, Wr["p"][:, k, 512:768],
                                     start=(k == 0), stop=(k == KS - 1))
                # split adds + DMAs per half so the final transfer is small
                ot = opool.tile([P, C], f32, tag="ot", name="ot")
                nc.vector.tensor_tensor(ot[:, 0:512], po_[0][:], bpB[:, 0:512],
                                        op=ALU.add)
                nc.sync.dma_start(y_d[ts_ * P:(ts_ + 1) * P, 0:512],
                                  ot[:, 0:512])
                nc.vector.tensor_tensor(ot[:, 512:768], po_[1][:, 0:256],
                                        bpB[:, 512:768], op=ALU.add)
                nc.sync.dma_start(y_d[ts_ * P:(ts_ + 1) * P, 512:768],
                                  ot[:, 512:768])

        if reps == 1:
            body()
        else:
            import concourse.mybir as _mb
            with tc.For_i(0, reps, 1, hint_engines=tuple(_mb.ALL_ENGINES)):
                body()

    nc.compile()
    return nc


class _Runner:
    """Compile once, run many times on the 8 axon-tunneled cores via PJRT."""

    def __init__(self, nc, n_cores):
        import jax
        import concourse.mybir as mybir
        from jax.sharding import Mesh, PartitionSpec
        from jax.experimental.shard_map import shard_map
        from concourse.bass2jax import (
            _bass_exec_p, install_neuronx_cc_hook, partition_id_tensor)

        install_neuronx_cc_hook()
        self.jax = jax
        self.n_cores = n_cores
        partition_name = nc.partition_id_tensor.name if nc.partition_id_tensor else None
        in_names, out_names, out_avals, zero_outs = [], [], [], []
        for alloc in nc.m.functions[0].allocations:
            if not isinstance(alloc, mybir.MemoryLocationSet):
                continue
            name = alloc.memorylocations[0].name
            if alloc.kind == "ExternalInput":
                if name != partition_name:
                    in_names.append(name)
            elif alloc.kind == "ExternalOutput":
                shape = tuple(alloc.tensor_shape)
                dtype = mybir.dt.np(alloc.dtype)
                out_names.append(name)
                out_avals.append(jax.core.ShapedArray(shape, dtype))
                zero_outs.append(np.zeros(shape, dtype))
        self.in_names, self.out_names = in_names, out_names
        self.zero_outs = zero_outs
        all_in = list(in_names) + list(out_names)
        if partition_name is not None:
            all_in.append(partition_name)

        def _body(*args):
            operands = list(args)
            if partition_name is not None:
                operands.append(partition_id_tensor())
            return tuple(_bass_exec_p.bind(
                *operands, out_avals=tuple(out_avals), in_names=tuple(all_in),
                out_names=tuple(out_names), lowering_input_output_aliases=(),
                sim_require_finite=True, sim_require_nnan=True, nc=nc))

        devices = jax.devices()[:n_cores]
        self.mesh = Mesh(np.asarray(devices), ("core",))
        spec = PartitionSpec("core")
        self.fn = jax.jit(
            shard_map(_body, mesh=self.mesh,
                      in_specs=(spec,) * (len(in_names) + len(out_names)),
                      out_specs=(spec,) * len(out_names), check_rep=False),
            keep_unused=True)

    def stage(self, in_maps):
        import jax
        from jax.sharding import PartitionSpec
        concat = [
            np.concatenate([np.asarray(in_maps[c][n]) for c in range(self.n_cores)], axis=0)
            for n in self.in_names
        ] + [np.concatenate([z] * self.n_cores, axis=0) for z in self.zero_outs]
        sharding = jax.sharding.NamedSharding(self.mesh, PartitionSpec("core"))
        return [jax.device_put(a, sharding) for a in concat]

    def run(self, staged):
        outs = self.fn(*staged)
        self.jax.block_until_ready(outs)
        return outs

    def run_to_maps(self, staged):
        outs = self.run(staged)
        res = []
        for c in range(self.n_cores):
            m = {}
            for i, n in enumerate(self.out_names):
                g = np.asarray(outs[i])
                per = g.shape[0] // self.n_cores
                m[n] = g[c * per:(c + 1) * per]
            res.append(m)
        return res


def get_runner(reps: int = 1, phases: int = 4, variant: str = "full"):
    key = (reps, phases, variant)
    if key not in _RUNNER_CACHE:
        nc = build_nc(reps, phases, variant)
        _RUNNER_CACHE[key] = _Runner(nc, N_CORES)
    return _RUNNER_CACHE[key]


def make_in_maps(x, Wq, bq, Wk, bk, Wv, bv, Wp, bp):
    import ml_dtypes
    bf = ml_dtypes.bfloat16
    x = np.asarray(x, dtype=np.float32)
    weights = {
        "Wq": np.asarray(Wq, bf), "Wk": np.asarray(Wk, bf),
        "Wv": np.asarray(Wv, bf), "Wp": np.asarray(Wp, bf),
    }
    bqT = np.ascontiguousarray(np.asarray(bq, np.float32).reshape(KS, P).T)
    bkT = np.ascontiguousarray(np.asarray(bk, np.float32).reshape(KS, P).T)
    bvB = np.asarray(bv, np.float32).reshape(1, C).copy()
    bpB = np.ascontiguousarray(np.broadcast_to(np.asarray(bp, np.float32), (P, C)))
    in_maps = []
    for b in range(B):
        in_maps.append({
            "xT": np.ascontiguousarray(x[b].T.astype(bf)),
            "Wq": weights["Wq"], "Wk": weights["Wk"],
            "Wv": weights["Wv"], "Wp": weights["Wp"],
            "bqT": bqT, "bkT": bkT, "bvB": bvB, "bpB": bpB,
        })
    return in_maps


def kernel(x, Wq, bq, Wk, bk, Wv, bv, Wp, bp):
    runner = get_runner(reps=1)
    in_maps = make_in_maps(x, Wq, bq, Wk, bk, Wv, bv, Wp, bp)
    staged = runner.stage(in_maps)
    res = runner.run_to_maps(staged)
    return np.stack([res[b]["y"] for b in range(B)], axis=0)
